# revision 1
# baseline (speedup 1.0000x reference)
"""MoE pointwise conv2d kernel for Trainium2 (8 NeuronCores, SPMD data-parallel).

Problem: out[b,o,h,w] = sum_i (sum_e routing[b,e] * weight[e,o,i]) * x[b,i,h,w]
Shapes:  x [64,384,28,28] f32, routing [64,8] f32, weight [8,384,384] f32.

Strategy (per core, 8 samples each), fp16 wire format end-to-end:
  - Routing-combine runs on TensorE (measured DVE scalar_tensor_tensor runs
    1x-mode only => a DVE MAC chain costs ~71us/core; TensorE does the same
    contraction in ~8us):
      The host expands routing into a sparse matrix
        rq[(e,o16), (b,o16')] = r[b,e] * delta(o16,o16')   [128 x 128]
      and pre-permutes weights to
        wt[(e,o16), (ki, chunk, i_lo)]                     [128 x 9216]
      so one matmul per (ki, o-chunk of 16) computes
        agg^T[i_lo, (b, o16)] = sum_e r[b,e] w[e, chunk*16+o16, ki*128+i_lo]
      for ALL 8 samples at once: 72 matmuls, FD=128, fp32 PSUM accumulate.
  - ScalarE evacuates agg psum tiles ([128,512], 4 chunks each) into a
    [128, 9216] f16 staging tile laid out (ki, chunk, b, o16).
  - Main GEMM out[b] = agg_b @ x_b on TensorE: lhsT tiles are strided 3D APs
    into staging (8 chunks x 16 cols per (ki,mo,b)); psum [128,784] spanning
    2 banks, accumulation groups FD 512 + 272 over 3 k-tiles.
  - PSUM out evacuation alternates ScalarE/VectorE; one [128, 3*784] out
    tile per sample.
  - DMAs per rep: 6 wt (split by ki x 2) + 1 rq + 8 x + 10 out = 25
    (last sample's out DMA is split per-mo to trim the tail); merged
    multi-dim access patterns keep the ~0.6us/DMA HWDGE+SP dispatch cost
    off the critical path (the 2-DMA-per-tile layout had 81).
"""
import os
import sys

sys.path.insert(0, "/opt/trn_rl_repo")

import numpy as np
from contextlib import ExitStack

B, C_IN, C_OUT, E, H, W = 64, 384, 384, 8, 28, 28
HW = H * W            # 784
N_CORES = 8
BPC = B // N_CORES    # 8 samples per core
KI = C_IN // 128      # 3 k-tiles
MO = C_OUT // 128     # 3 output-partition tiles
OC = 16               # o-values per chunk
NCH = C_OUT // OC     # 24 o-chunks
CPK = NCH * 128       # staging cols per ki (3072)
SCOL = KI * CPK       # staging cols total (9216)
NSPLITS = ((0, 512), (512, 272))  # psum accumulation groups (bank-aligned)

_cache = {}


def _build(reps=1, serialize_reps=False, small_out=False, cg4=4,
           evac_split=True, wt_splits=2, out_split_mo=False,
           agg_evac_split=False, psm_split=False, deep_bufs=4,
           agg_evac_pair=False, wt_head=True, agg_order="ki",
           main_hybrid=0, x_pair=False, share_psum=True, ki_major=False):
    import concourse.tile as tile
    import concourse.mybir as mybir
    from concourse import bacc
    from concourse.tile import add_dep_helper

    f32 = mybir.dt.float32
    f16 = mybir.dt.float16

    nc = bacc.Bacc("TRN2", target_bir_lowering=False, debug=False)
    x_d = nc.dram_tensor("x", [BPC, KI, 128, HW], f16, kind="ExternalInput")
    rq_d = nc.dram_tensor("rq", [128, 128], f16, kind="ExternalInput")
    wt_d = nc.dram_tensor("wt", [KI, 128, CPK], f16, kind="ExternalInput")
    out_d = nc.dram_tensor("out", [(1 if small_out else reps) * BPC, MO, 128, HW],
                           f16, kind="ExternalOutput")

    with tile.TileContext(nc) as tc:
        with ExitStack() as ctx:
            wt_pool = ctx.enter_context(tc.tile_pool(name="wt", bufs=2))
            rq_pool = ctx.enter_context(tc.tile_pool(name="rq", bufs=2))
            stag_pool = ctx.enter_context(tc.tile_pool(name="st", bufs=2))
            nbuf = deep_bufs if isinstance(deep_bufs, int) and deep_bufs > 1 \
                else (4 if deep_bufs else 3)
            x_pool = ctx.enter_context(tc.tile_pool(name="xp", bufs=nbuf))
            out_pool = ctx.enter_context(tc.tile_pool(name="op", bufs=nbuf))
            psm_pool = ctx.enter_context(tc.tile_pool(
                name="pm", bufs=4 if share_psum else 3, space="PSUM"))
            psa_pool = psm_pool if share_psum else ctx.enter_context(
                tc.tile_pool(name="pa", bufs=2, space="PSUM"))
            psm2_pool = ctx.enter_context(tc.tile_pool(
                name="pm2", bufs=3, space="PSUM")) if psm_split else None

            prev_out_dmas, cur_out_dmas = [], []

            def _fence(inst):
                if serialize_reps:
                    for d in prev_out_dmas:
                        add_dep_helper(inst.ins, d.ins, reason="serialize reps")
                return inst

            for rep in range(reps):
                prev_out_dmas, cur_out_dmas = cur_out_dmas, []
                rq_sb = rq_pool.tile([128, 128], f16)
                _fence(nc.sync.dma_start(rq_sb[:], rq_d[:]))
                wt_sb = wt_pool.tile([128, SCOL], f16)
                wt_dmas = []
                pieces = []
                if agg_order == "cg":
                    # 512-col pieces in (cg, ki) order: each piece lands just
                    # before the agg matmul group that consumes it
                    psz = cg4 * 128
                    for s in range(CPK // psz):
                        for ki in range(KI):
                            pieces.append((ki, s * psz, (s + 1) * psz))
                else:
                    csz = CPK // wt_splits
                    for ki in range(KI):
                        lo = 0
                        if wt_head and ki == 0:
                            pieces.append((0, 0, 512))
                            lo = 512
                        for s in range(wt_splits):
                            hi = (s + 1) * csz
                            if hi > lo:
                                pieces.append((ki, lo, hi))
                                lo = hi
                for ki, lo, hi in pieces:
                    wt_dmas.append(_fence(nc.sync.dma_start(
                        wt_sb[:, ki * CPK + lo:ki * CPK + hi],
                        wt_d[ki, :, lo:hi])))

                # ---- routing-combine on TensorE ----
                # stag[(ki, chunk, o16, b)] = agg[b, chunk*16+o16, ki*128+p]
                stag = stag_pool.tile([128, SCOL], f16)
                if agg_order == "cg":
                    order = [(ki, cg) for cg in range(NCH // cg4)
                             for ki in range(KI)]
                else:
                    order = [(ki, cg) for ki in range(KI)
                             for cg in range(NCH // cg4)]
                for ki, cg in order:
                    if True:
                        ps = psa_pool.tile(
                            [128, HW if share_psum else cg4 * 128], f32,
                            tag="ps")
                        for c4 in range(cg4):
                            chunk = cg * cg4 + c4
                            nc.tensor.matmul(
                                ps[:, c4 * 128:(c4 + 1) * 128],
                                wt_sb[:, (ki * NCH + chunk) * 128:
                                      (ki * NCH + chunk + 1) * 128],
                                rq_sb[:],
                                start=True, stop=True,
                            )
                        base = (ki * NCH + cg * cg4) * 128
                        half = cg4 * 128 // 2
                        if agg_evac_pair:
                            nc.scalar.copy(stag[:, base:base + half],
                                           ps[:, 0:half])
                            nc.vector.tensor_copy(
                                stag[:, base + half:base + cg4 * 128],
                                ps[:, half:cg4 * 128])
                        elif agg_evac_split and cg % 2 == 1:
                            nc.vector.tensor_copy(
                                stag[:, base:base + cg4 * 128],
                                ps[:, 0:cg4 * 128])
                        else:
                            nc.scalar.copy(stag[:, base:base + cg4 * 128],
                                           ps[:, 0:cg4 * 128])

                # ---- per-sample GEMM + evac + out DMA ----
                # First `main_hybrid` samples run mo-interleaved so early
                # units only need the agg evacs that have already drained.
                H = min(main_hybrid, BPC)
                units = ([(b, mo) for mo in range(MO) for b in range(H)]
                         + [(b, mo) for b in range(H, BPC)
                            for mo in range(MO)])
                x_sbs, o_sbs = {}, {}
                for b, mo in units:
                    if b not in x_sbs:
                        if x_pair:
                            b0 = b - b % 2
                            x_sb = x_pool.tile([128, 2, KI, HW], f16, tag="x")
                            xi = _fence(nc.sync.dma_start(
                                x_sb[:],
                                x_d[b0:b0 + 2].transpose([2, 0, 1, 3])))
                            x_sbs[b0] = x_sb[:, 0]
                            x_sbs[b0 + 1] = x_sb[:, 1]
                        else:
                            x_sb = x_pool.tile([128, KI, HW], f16, tag="x")
                            xi = _fence(nc.sync.dma_start(
                                x_sb[:], x_d[b].transpose([1, 0, 2])))
                            x_sbs[b] = x_sb
                        if b < 2:
                            for wd in wt_dmas:
                                add_dep_helper(xi.ins, wd.ins,
                                               reason="x after wt (head trim)")
                    if b not in o_sbs:
                        o_sb = out_pool.tile([128, MO, HW], f16, tag="o")
                        o_sbs[b] = o_sb
                    x_sb, o_sb = x_sbs[b], o_sbs[b]
                    if True:
                        if psm_split:
                            ps_a = psm_pool.tile([128, 512], f32)
                            ps_b = psm2_pool.tile([128, HW - 512], f32)
                            segs = ((0, 512, ps_a), (512, HW - 512, ps_b))
                        else:
                            ps = psm_pool.tile([128, HW], f32, tag="ps")
                            segs = tuple((n0, nw, ps[:, n0:n0 + nw])
                                         for n0, nw in NSPLITS)
                        mm_order = (
                            [(ki, s) for ki in range(KI) for s in segs]
                            if ki_major else
                            [(ki, s) for s in segs for ki in range(KI)])
                        for ki, (n0, nw, pseg) in mm_order:
                            base = (ki * NCH + mo * (NCH // MO)) * 128
                            lhs = stag[:, base + b:base + 1024:BPC]
                            nc.tensor.matmul(
                                pseg[:] if psm_split else pseg,
                                lhs, x_sb[:, ki, n0:n0 + nw],
                                start=(ki == 0), stop=(ki == KI - 1),
                            )
                        if psm_split:
                            eng = (nc.vector.tensor_copy
                                   if evac_split and mo >= 1 else nc.scalar.copy)
                            eng(o_sb[:, mo, 0:512], ps_a[:])
                            eng(o_sb[:, mo, 512:HW], ps_b[:])
                        elif evac_split and mo >= 1:
                            nc.vector.tensor_copy(o_sb[:, mo, :], ps[:])
                        else:
                            nc.scalar.copy(o_sb[:, mo, :], ps[:])
                        if out_split_mo or b == BPC - 1:
                            cur_out_dmas.append(nc.sync.dma_start(
                                out_d[(0 if small_out else rep) * BPC + b,
                                      mo], o_sb[:, mo, :]))
                    if mo == MO - 1 and not (out_split_mo or b == BPC - 1):
                        cur_out_dmas.append(nc.sync.dma_start(
                            out_d[(0 if small_out else rep) * BPC + b]
                            .transpose([1, 0, 2]), o_sb[:]))
    nc.compile()
    return nc


def _host_prep(x, routing_weights, weight):
    """Full inputs -> per-core in_maps with the kernel's dram layouts."""
    # wt[ki][e*16+o16, chunk*128 + i_lo] = weight[e, chunk*16+o16, ki*128+i_lo]
    wt = np.ascontiguousarray(
        weight.reshape(E, NCH, OC, KI, 128)      # e, chunk, o16, ki, i_lo
        .transpose(3, 0, 2, 1, 4)                # ki, e, o16, chunk, i_lo
        .reshape(KI, 128, CPK).astype(np.float16))
    x_r = np.ascontiguousarray(x.reshape(B, KI, 128, HW).astype(np.float16))

    in_maps = []
    for c in range(N_CORES):
        r_core = routing_weights[c * BPC:(c + 1) * BPC]   # [BPC, E]
        rq = np.zeros((E, OC, OC, BPC), dtype=np.float16)
        for o16 in range(OC):
            rq[:, o16, o16, :] = r_core.T.astype(np.float16)
        in_maps.append({
            "x": x_r[c * BPC:(c + 1) * BPC],
            "rq": np.ascontiguousarray(rq.reshape(128, 128)),
            "wt": wt,
        })
    return in_maps


def kernel(x: np.ndarray, routing_weights: np.ndarray, weight: np.ndarray,
           _trace: bool = False):
    from concourse.bass_utils import run_bass_kernel_spmd

    x = np.asarray(x, dtype=np.float32)
    routing_weights = np.ascontiguousarray(
        np.asarray(routing_weights, dtype=np.float32))
    weight = np.asarray(weight, dtype=np.float32)

    if "nc" not in _cache:
        _cache["nc"] = _build()
    nc = _cache["nc"]

    in_maps = _host_prep(x, routing_weights, weight)
    res = run_bass_kernel_spmd(nc, in_maps, core_ids=list(range(N_CORES)),
                               trace=_trace)
    out = np.concatenate([res.results[c]["out"] for c in range(N_CORES)],
                         axis=0)
    if _trace:
        _cache["last_result"] = res
    return out.reshape(B, C_OUT, H, W).astype(np.float32)


if __name__ == "__main__":
    rng = np.random.default_rng(0)
    x = rng.standard_normal((B, C_IN, H, W), dtype=np.float32)
    rw = rng.random((B, E), dtype=np.float32)
    w = rng.standard_normal((E, C_OUT, C_IN), dtype=np.float32)
    got = kernel(x, rw, w)
    agg = np.einsum('be,eoi->boi', rw, w)
    want = np.einsum('boi,bihw->bohw', agg, x.reshape(B, C_IN, H, W))
    err = np.abs(got - want).max() / np.abs(want).max()
    print("rel err:", err)



# revision 20
# speedup vs baseline: 1.7048x; 1.7048x over previous
"""MoE pointwise conv2d kernel for Trainium2 (8 NeuronCores, SPMD data-parallel).

Problem: out[b,o,h,w] = sum_i (sum_e routing[b,e] * weight[e,o,i]) * x[b,i,h,w]
Shapes:  x [64,384,28,28] f32, routing [64,8] f32, weight [8,384,384] f32.

v2 design (per core, 8 samples). PE floor is 65,664 cycles (56,448 main GEMM
+ 9,216 routing-combine) = 27.4us @ 2.4GHz; the v1 kernel measured 41.7us
because the combine phase was wt-DMA starved, the main phase was PSUM-evac
rate limited, and ~7us of tail DMAs serialized. Fixes:

  - Wire formats: x and wt ship as fp8-e3m4 (x*2, wt*32, with 1/64 folded
    into the f16 rq matrix so no on-chip rescale is needed). Exact end-to-end
    rel-err measured vs the harness inputs: 1.84e-2 (gate 2e-2); f16 staging
    and f16 out keep the rest of the error budget. Halves x+wt DMA bytes:
    total DMA 23.8us/rep < PE 27.4us, so DMA comes off the critical path.
  - Single interleaved PE stream: combine work is chopped into 6 chunk-groups
    (cg) of 12 matmuls; main-GEMM units u(b,mo) are interleaved so the PE
    never waits on a wt piece; per-mo staging tiles give the tile framework
    exact producer->consumer deps.
  - PE p-state warmup: 7 throwaway FD-512 matmuls (no DMA deps) run during
    the ~3us DMA head, so the clock is at 2.4GHz when real work starts
    (ramp model: 1.2GHz until 3us of continuous busy).
  - PSUM per unit is split A=[0:512), B=[512:784) (bank-aligned accumulation
    groups); A evacuates on one engine while B's matmuls run, B on the other
    engine; assignments alternate so ScalarE/DVE each stay ~60% busy.
  - Tail: the last unit's out DMA is split A/B so the final DMA covers only
    272 columns; tail = evac(272) + DMA pipe ~= 2.9us.
"""
import os
import sys

sys.path.insert(0, "/opt/trn_rl_repo")

import numpy as np
from contextlib import ExitStack

B, C_IN, C_OUT, E, H, W = 64, 384, 384, 8, 28, 28
HW = H * W            # 784
N_CORES = 8
BPC = B // N_CORES    # 8 samples per core
KI = C_IN // 128      # 3 k-tiles
MO = C_OUT // 128     # 3 output-partition tiles
OC = 16               # o-values per chunk
NCH = C_OUT // OC     # 24 o-chunks
CG4 = 4               # chunks per combine group
NCG = NCH // CG4      # 6 combine groups (2 per mo block)
WTC = KI * CG4 * 128  # wt cols per cg tile (1536)
STC = KI * CG4 * 2 * 128  # staging cols per mo tile (3072)
ASPL = 448            # main psum A split (fits one 2KB bank)
BSPL = HW - ASPL      # main psum B split (272)
X_SCALE = 2.0
WT_SCALE = 32.0

_cache = {}


def _build(reps=1, serialize_reps=False, warm_mms=7, small_out=False,
           in_q='sp-interleave', cg_order='spread', out_q='sp-poolfin',
           agg_bufs=4, a_bufs=2, b_bufs=2, comb_evac='ki0act'):
    import concourse.tile as tile
    import concourse.mybir as mybir
    from concourse import bacc
    from concourse.tile import add_dep_helper

    f32 = mybir.dt.float32
    f16 = mybir.dt.float16
    f8 = mybir.dt.float8e3

    nc = bacc.Bacc("TRN2", target_bir_lowering=False, debug=False)
    x_d = nc.dram_tensor("x", [BPC, KI, 128, HW], f8, kind="ExternalInput")
    rq_d = nc.dram_tensor("rq", [128, 128], f16, kind="ExternalInput")
    wt_d = nc.dram_tensor("wt", [NCG, 128, WTC], f8, kind="ExternalInput")
    out_d = nc.dram_tensor("out", [(1 if small_out else reps) * BPC, MO, 128, HW],
                           f16, kind="ExternalOutput")

    with tile.TileContext(nc) as tc:
        with ExitStack() as ctx:
            warm_pool = ctx.enter_context(tc.tile_pool(name="wm", bufs=1))
            wt_pool = ctx.enter_context(tc.tile_pool(name="wt", bufs=NCG))
            rq_pool = ctx.enter_context(tc.tile_pool(name="rq", bufs=2))
            stag_pool = ctx.enter_context(tc.tile_pool(name="st", bufs=MO))
            x_pool = ctx.enter_context(tc.tile_pool(name="xp", bufs=BPC))
            out_pool = ctx.enter_context(tc.tile_pool(name="op", bufs=6))
            # PSUM budget: agg + A + B tiles must fit 8 x 2KB banks
            agg_pool = ctx.enter_context(tc.tile_pool(name="pa", bufs=agg_bufs,
                                                      space="PSUM"))
            psa_pool = ctx.enter_context(tc.tile_pool(name="pA", bufs=a_bufs,
                                                      space="PSUM"))
            psb_pool = ctx.enter_context(tc.tile_pool(name="pB", bufs=b_bufs,
                                                      space="PSUM"))

            prev_out_dmas, cur_out_dmas = [], []

            def _fence(inst):
                if serialize_reps:
                    for d in prev_out_dmas:
                        add_dep_helper(inst.ins, d.ins, reason="serialize reps")
                return inst

            # warmup source: one zeroed tile shared by every rep's warmup
            warm = warm_pool.tile([128, ASPL], f16, tag="wm")
            nc.vector.memset(warm[:], 0.0)

            for rep in range(reps):
                prev_out_dmas, cur_out_dmas = cur_out_dmas, []

                # ---- input DMAs. All SP/Act DMA generations funnel through
                # ONE shared HWDGE (625ns each), so early x loads go via
                # Pool's separate SWDGE generator instead.
                # SP/HWDGE: rq, wt0 (2 pieces), wt1, x4..x7, then out DMAs.
                # Act/HWDGE: wt2..wt5 (configs done before evacs begin).
                # Pool/SWDGE: x0..x3, then mid out DMAs.
                rq_sb = rq_pool.tile([128, 128], f16)
                _fence(nc.sync.dma_start(rq_sb[:], rq_d[:]))
                wt_sbs, x_sbs = {}, {}

                def load_wt_piece(cg, eng, lo, hi):
                    if cg not in wt_sbs:
                        wt_sbs[cg] = wt_pool.tile([128, WTC], f8, tag="wt",
                                                  name=f"wt{cg}")
                    _fence(eng.dma_start(wt_sbs[cg][:, lo:hi],
                                         wt_d[cg][:, lo:hi]))

                def load_wt_pair(cg, eng):
                    # one DMA covering cgs cg and cg+1 (contiguous in dram)
                    pair = wt_pool.tile([128, 2, WTC], f8, tag="wt",
                                        name=f"wtp{cg}")
                    wt_sbs[cg] = pair[:, 0]
                    wt_sbs[cg + 1] = pair[:, 1]
                    _fence(eng.dma_start(
                        pair[:], wt_d[cg:cg + 2].transpose([1, 0, 2])))

                def load_x(b, eng):
                    x_sbs[b] = x_pool.tile([128, KI, HW], f8, tag="x",
                                           name=f"xs{b}")
                    _fence(eng.dma_start(
                        x_sbs[b][:], x_d[b].transpose([1, 0, 2])))

                if in_q == 'deadline':
                    load_wt_piece(0, nc.sync, 0, 512)
                    load_x(0, nc.gpsimd)
                    load_wt_piece(0, nc.sync, 512, WTC)
                    load_x(1, nc.gpsimd)
                    load_wt_piece(1, nc.sync, 0, WTC)
                    load_x(2, nc.gpsimd)
                    load_wt_pair(2, nc.sync)
                    load_x(3, nc.gpsimd)
                    load_wt_pair(4, nc.gpsimd)
                    for b in range(4, BPC):
                        load_x(b, nc.sync)
                elif in_q == 'sp-all':
                    for cg in range(NCG):
                        load_wt_piece(cg, nc.sync, 0, WTC)
                    for b in range(BPC):
                        load_x(b, nc.sync)
                elif in_q == 'sp-interleave':
                    load_wt_piece(0, nc.sync, 0, 512)
                    load_wt_piece(0, nc.sync, 512, WTC)
                    load_wt_piece(1, nc.sync, 0, WTC)
                    load_x(0, nc.sync)
                    load_wt_piece(2, nc.sync, 0, WTC)
                    load_x(1, nc.sync)
                    load_wt_piece(3, nc.sync, 0, WTC)
                    load_x(2, nc.sync)
                    load_wt_piece(4, nc.sync, 0, WTC)
                    load_wt_piece(5, nc.sync, 0, WTC)
                    for b in range(3, BPC):
                        load_x(b, nc.sync)

                # ---- PE p-state warmup (no DMA deps beyond the rep fence) --
                if warm_mms:
                    wps = agg_pool.tile([128, ASPL], f32, tag="ps")
                    for w_i in range(warm_mms):
                        _fence(nc.tensor.matmul(wps[:], warm[:, 0:128],
                                                warm[:], start=True,
                                                stop=True))

                # ---- staging tiles (one per mo block) ----
                stags = [stag_pool.tile([128, STC], f16, tag="st",
                                        name=f"stag{m}")
                         for m in range(MO)]

                evac_flip = [0]

                def combine_group(cg):
                    # 12 matmuls: agg^T[i_lo, (o16,b)] for chunks cg*4..+4
                    mo, cgin = divmod(cg, 2)
                    for ki in range(KI):
                        ps = agg_pool.tile([128, CG4 * 128], f32, tag="ps")
                        for c4 in range(CG4):
                            nc.tensor.matmul(
                                ps[:, c4 * 128:(c4 + 1) * 128],
                                wt_sbs[cg][:, (ki * CG4 + c4) * 128:
                                           (ki * CG4 + c4 + 1) * 128],
                                rq_sb[:],
                                start=True, stop=True,
                            )
                        dst = stags[mo][:, ki * 1024 + cgin * 512:
                                        ki * 1024 + cgin * 512 + 512]
                        if comb_evac == 'ki0act':
                            use_act = (ki == 0)
                        elif comb_evac == 'alt':
                            use_act = (evac_flip[0] % 2 == 0)
                        else:  # '2act'
                            use_act = (ki != 1)
                        evac_flip[0] += 1
                        if use_act:
                            nc.scalar.copy(dst, ps[:])
                        else:
                            nc.vector.tensor_copy(dst, ps[:])

                unit_idx = [0]

                def main_unit(b, mo, last=False):
                    ps_a = psa_pool.tile([128, ASPL], f32, tag="pA")
                    ps_b = psb_pool.tile([128, BSPL], f32, tag="pB")
                    x_sb = x_sbs[b]
                    for n0, nw, ps in ((0, ASPL, ps_a), (ASPL, BSPL, ps_b)):
                        for ki in range(KI):
                            lhs = stags[mo][:, ki * 1024 + b:
                                            ki * 1024 + 1024:BPC]
                            nc.tensor.matmul(
                                ps[:, 0:nw], lhs, x_sb[:, ki, n0:n0 + nw],
                                start=(ki == 0), stop=(ki == KI - 1),
                            )
                    o_sb = out_pool.tile([128, HW], f16, tag="o")
                    ob = (0 if small_out else rep) * BPC + b
                    # A (512, on the faster ScalarE) overlaps B's matmuls;
                    # B (272) on DVE. Both fit under the 980ns unit cadence.
                    nc.scalar.copy(o_sb[:, 0:ASPL], ps_a[:])
                    nc.vector.tensor_copy(o_sb[:, ASPL:HW], ps_b[:])
                    if out_q == 'sp':
                        dma_eng = fin_a = fin_b = nc.sync
                    elif out_q == 'mixed':
                        dma_eng = nc.sync if (unit_idx[0] < 12 or
                                              unit_idx[0] >= 21) else nc.gpsimd
                        fin_a, fin_b = nc.sync, nc.gpsimd
                    elif out_q == 'mixed-act':
                        dma_eng = nc.sync if (unit_idx[0] < 12 or
                                              unit_idx[0] >= 21) else nc.gpsimd
                        fin_a, fin_b = nc.scalar, nc.scalar
                    elif out_q == 'sp-poolfin':
                        dma_eng = nc.sync
                        fin_a, fin_b = nc.gpsimd, nc.sync
                    unit_idx[0] += 1
                    if last:
                        cur_out_dmas.append(fin_a.dma_start(
                            out_d[ob, mo][:, 0:ASPL], o_sb[:, 0:ASPL]))
                        cur_out_dmas.append(fin_b.dma_start(
                            out_d[ob, mo][:, ASPL:HW], o_sb[:, ASPL:HW]))
                    else:
                        cur_out_dmas.append(dma_eng.dma_start(
                            out_d[ob, mo], o_sb[:]))

                # ---- interleaved PE schedule ----
                if cg_order == 'spread':
                    sched = [('c', 0), ('c', 1), ('u', 0, 0), ('u', 1, 0),
                             ('u', 2, 0), ('u', 3, 0), ('c', 2), ('u', 4, 0),
                             ('u', 5, 0), ('c', 3), ('u', 6, 0), ('u', 7, 0),
                             ('u', 0, 1), ('u', 1, 1), ('c', 4), ('u', 2, 1),
                             ('u', 3, 1), ('c', 5)] +                             [('u', b, 1) for b in range(4, BPC)] +                             [('u', b, 2) for b in range(BPC)]
                elif cg_order == 'front':
                    sched = [('c', 0), ('c', 1), ('c', 2), ('u', 0, 0),
                             ('c', 3), ('u', 1, 0), ('c', 4), ('u', 2, 0),
                             ('c', 5)] +                             [('u', b, 0) for b in range(3, BPC)] +                             [('u', b, 1) for b in range(BPC)] +                             [('u', b, 2) for b in range(BPC)]
                for step in sched:
                    if step[0] == 'c':
                        combine_group(step[1])
                    else:
                        _, b, mo = step
                        main_unit(b, mo, last=(mo == MO - 1 and b == BPC - 1))
    nc.compile()
    return nc


def _host_prep(x, routing_weights, weight):
    """Full inputs -> per-core in_maps with the kernel's dram layouts."""
    import ml_dtypes
    f8 = ml_dtypes.float8_e3m4

    # wt[cg][(e,o16)][(ki, c4, i_lo)] = weight[e, (cg*4+c4)*16+o16, ki*128+i_lo]
    wt = np.ascontiguousarray(
        (weight * WT_SCALE)
        .reshape(E, NCG, CG4, OC, KI, 128)   # e, cg, c4, o16, ki, i_lo
        .transpose(1, 0, 3, 4, 2, 5)         # cg, e, o16, ki, c4, i_lo
        .reshape(NCG, 128, WTC).astype(f8))
    x_r = np.ascontiguousarray(
        (x * X_SCALE).reshape(B, KI, 128, HW).astype(f8))

    in_maps = []
    for c in range(N_CORES):
        r_core = routing_weights[c * BPC:(c + 1) * BPC]   # [BPC, E]
        rq = np.zeros((E, OC, OC, BPC), dtype=np.float16)
        for o16 in range(OC):
            rq[:, o16, o16, :] = (r_core.T / (X_SCALE * WT_SCALE)).astype(
                np.float16)
        in_maps.append({
            "x": x_r[c * BPC:(c + 1) * BPC],
            "rq": np.ascontiguousarray(rq.reshape(128, 128)),
            "wt": wt,
        })
    return in_maps


def kernel(x: np.ndarray, routing_weights: np.ndarray, weight: np.ndarray,
           _trace: bool = False):
    from concourse.bass_utils import run_bass_kernel_spmd

    x = np.asarray(x, dtype=np.float32)
    routing_weights = np.ascontiguousarray(
        np.asarray(routing_weights, dtype=np.float32))
    weight = np.asarray(weight, dtype=np.float32)

    if "nc" not in _cache:
        _cache["nc"] = _build()
    nc = _cache["nc"]

    in_maps = _host_prep(x, routing_weights, weight)
    res = run_bass_kernel_spmd(nc, in_maps, core_ids=list(range(N_CORES)),
                               trace=_trace)
    out = np.concatenate([res.results[c]["out"] for c in range(N_CORES)],
                         axis=0)
    if _trace:
        _cache["last_result"] = res
    return out.reshape(B, C_OUT, H, W).astype(np.float32)


if __name__ == "__main__":
    rng = np.random.default_rng(0)
    x = rng.standard_normal((B, C_IN, H, W), dtype=np.float32)
    rw = rng.random((B, E), dtype=np.float32)
    w = rng.standard_normal((E, C_OUT, C_IN), dtype=np.float32)
    got = kernel(x, rw, w)
    agg = np.einsum('be,eoi->boi', rw, w)
    want = np.einsum('boi,bihw->bohw', agg, x.reshape(B, C_IN, H, W))
    err = np.abs(got - want).max() / np.abs(want).max()
    print("rel err:", err)
